# revision 37
# baseline (speedup 1.0000x reference)
"""Mixtral-style GQA attention block on 8 Trainium2 NeuronCores.

Tensor-parallel over heads: core c owns q-heads [4c..4c+4) and kv-head c.
All matmuls run in bf16 (1 cycle/row on the PE vs fp32r's 2-pass mode);
accumulation stays fp32 in PSUM. Numerics check: end-to-end bf16 gives
rel_l2 ~8e-3 vs the fp32 reference (gate is 2e-2).

Pipeline per core:
  qkv proj (bf16, full-32-chunk PSUM accumulation; k/v slots fused into
  one hid-arrival-paced loop to hide the 16MB activation load)
  -> RoPE (rotate-half via SBUF partition-swap DMA + DVE muls, no PE)
  -> causal attention in token-chunk order 1,2,3,0 (transposed-scores
     layout; softmax denominator accumulated on the DVE; per-chunk bf16
     AllGather triggered two pipeline steps after each chunk)
  -> o_proj per chunk at the tail; the list scheduler slots its matmuls
     into attention's scalar-exp bubbles, and the tail chain hangs off
     the smallest chunk so the last AllGather hides under o_proj work.
ag tiles share the attention tile pool so their AG-blocked DMAs can't
head-of-line-block the in-order sync engine ahead of latency-critical
norm DMAs. Host concatenates the per-core column slices.

Model dims (hardcoded): T=2048, HIDDEN=4096, H=32, KV=8, D=128.
"""

from contextlib import ExitStack

import ml_dtypes
import numpy as np

import concourse.bass_utils as _bu
import concourse.mybir as mybir
import concourse.tile as tile
from concourse import bacc
from concourse.bass_utils import run_bass_kernel_spmd

# ---- problem dims ----
T = 2048
HIDDEN = 4096
H = 32
KV = 8
D = 128
THETA = 10000.0
SCALE = D ** -0.5

CORES = 8
QH = H // CORES            # 4 q heads per core
SLOTS = QH + 2             # k, v, q0..q3 head-major slots
LOCAL = QH * D             # 512: per-core attention output dims
P = 128
NCH = T // 512             # 4 token chunks of 512
KCH = HIDDEN // P          # 32 contraction chunks
TT = T // P                # 16 token tiles of 128

F32 = mybir.dt.float32
F32R = mybir.dt.float32r
BF = mybir.dt.bfloat16
EXP = mybir.ActivationFunctionType.Exp
BF_NP = ml_dtypes.bfloat16


def build_nc():
    nc = bacc.Bacc(num_devices=CORES)

    # ---- per-core I/O (host pre-packs bf16 + stationary-major weights) ----
    hidT = nc.declare_dram_parameter("hidT", [HIDDEN, T], BF, isOutput=False)
    # w_qkvT[m*128+p, kc*128+c] = W_slot_m[c-th out row, kc*128+p]
    w_qkvT = nc.declare_dram_parameter("w_qkvT", [SLOTS * P, KCH * P], BF,
                                       isOutput=False)
    w_oT = nc.declare_dram_parameter("w_oT", [QH * P, KCH * P], BF,
                                     isOutput=False)
    cosT = nc.declare_dram_parameter("cosT", [P, T], BF, isOutput=False)
    sinT = nc.declare_dram_parameter("sinT", [P, T], BF, isOutput=False)
    outT = nc.declare_dram_parameter("outT", [LOCAL, T], F32, isOutput=True)

    # ---- consts ----
    tri_c = nc.inline_tensor(
        np.triu(np.ones((P, P), dtype=np.float32)).astype(BF_NP), name="tri_c")
    # dn lhsT for head h: [128, 4] with column h all-ones
    onc4 = np.zeros((P, QH, QH), dtype=np.float32)
    for h in range(QH):
        onc4[:, h, h] = 1.0
    onc4_c = nc.inline_tensor(
        np.ascontiguousarray(onc4.transpose(1, 0, 2)).astype(BF_NP),
        name="onc4_c")   # [QH, 128, 4]
    ones_row_c = nc.inline_tensor(
        np.ones((1, P), dtype=np.float32).astype(BF_NP), name="ones_row_c")

    # ---- collective bounce buffers (chunk-major, bf16) ----
    ag_in = nc.dram_tensor("ag_in", [NCH, LOCAL, 512], BF)
    ag_out = nc.dram_tensor("ag_out", [NCH, H * D, 512], BF,
                            addr_space="Shared")
    # tiny warmup collective: absorbs CC cold-start before AG(0)
    agw_in = nc.dram_tensor("agw_in", [1, 64], BF)
    agw_out = nc.dram_tensor("agw_out", [CORES, 64], BF, addr_space="Shared")
    # chunk 0 (the tail chunk) gathers in two 256-col halves so its o_proj
    # can start after the first half-flight
    agh_in = [nc.dram_tensor(f"agh_in{i}", [LOCAL, 256], BF) for i in (0, 1)]
    agh_out = [nc.dram_tensor(f"agh_out{i}", [H * D, 256], BF,
                              addr_space="Shared") for i in (0, 1)]

    with tile.TileContext(nc) as tc:
        with tc.tile_pool(name="const", bufs=1) as cpool:
            pstack = ExitStack()
            qpool = pstack.enter_context(tc.tile_pool(name="qk_out", bufs=1))

            # persistent bf16 attention operands
            q16 = [qpool.tile([P, T], BF, tag=f"q{h}", name=f"q{h}")
                   for h in range(QH)]
            k16 = qpool.tile([P, T], BF, tag="k", name="k")
            vtok = [qpool.tile([P, P], BF, tag=f"vt{j}", name=f"vt{j}")
                    for j in range(TT)]

            tri_sb = cpool.tile([P, P], BF, tag="tri")
            onc4_sb = [cpool.tile([P, QH], BF, tag=f"onc4_{h}",
                                  name=f"onc4_{h}") for h in range(QH)]
            onr_sb = cpool.tile([1, P], BF, tag="onr")

            # ============ phase 1: qkv proj + fused rope ====
            ph1 = ExitStack()
            hid_pool = ph1.enter_context(tc.tile_pool(name="hid", bufs=1))
            wq_pool = ph1.enter_context(tc.tile_pool(name="wq", bufs=1))
            cs_pool = ph1.enter_context(tc.tile_pool(name="cs", bufs=1))
            rp_pool = ph1.enter_context(tc.tile_pool(name="rp", bufs=1))
            pr_ps = ph1.enter_context(
                tc.tile_pool(name="pr_ps", bufs=1, space="PSUM"))

            # DMA issue order matters: the first matmul needs wm(m=0) and
            # ht[0], so those go first; everything else follows.
            wms = []
            for m in range(SLOTS):
                wm = wq_pool.tile([P, KCH * P], BF, tag="wm", name="wm",
                                  bufs=2)
                if m < 2:
                    nc.sync.dma_start(wm[:], w_qkvT[m * P:(m + 1) * P, :])
                wms.append(wm)
            hts = []
            for kc in range(KCH):
                ht = hid_pool.tile([P, T], BF, tag=f"hid{kc}", name="ht")
                nc.sync.dma_start(ht[:], hidT[kc * P:(kc + 1) * P, :])
                hts.append(ht)
            cos_sb = cs_pool.tile([P, T], BF, tag="cos")
            sin_sb = cs_pool.tile([P, T], BF, tag="sin")
            nc.sync.dma_start(cos_sb[:], cosT[:, :])
            nc.sync.dma_start(sin_sb[:], sinT[:, :])
            nc.sync.dma_start(tri_sb[:], tri_c[:, :])
            for h in range(QH):
                nc.sync.dma_start(onc4_sb[h][:], onc4_c[h])
            nc.sync.dma_start(onr_sb[:], ones_row_c[:, :])

            # warm up the collective path while qkv runs
            nc.gpsimd.collective_compute(
                "AllGather",
                mybir.AluOpType.bypass,
                replica_groups=[list(range(CORES))],
                ins=[agw_in[:]],
                outs=[agw_out[:]],
            )

            def rope_chunk(dst16, n, ps):
                """dst16[:, chunk n] = x*cos + rot(x)*sin (rot via DMA)."""
                t0 = n * 512
                xs = rp_pool.tile([P, 512], F32, tag="xs", name="xs", bufs=3)
                nc.scalar.copy(xs[:], ps[:])
                xr = rp_pool.tile([P, 512], F32, tag="xr", name="xr", bufs=3)
                nc.sync.dma_start(xr[0:64, :], xs[64:128, :])
                nc.sync.dma_start(xr[64:128, :], xs[0:64, :])
                tcos = rp_pool.tile([P, 512], F32, tag="tc", name="tc", bufs=3)
                nc.vector.tensor_mul(tcos[:], xs[:], cos_sb[:, t0:t0 + 512])
                tsin = rp_pool.tile([P, 512], F32, tag="ts", name="ts", bufs=3)
                nc.vector.tensor_mul(tsin[:], xr[:], sin_sb[:, t0:t0 + 512])
                nc.vector.tensor_add(dst16[:, t0:t0 + 512], tcos[:], tsin[:])

            # slot order: k, v, q0..q3 (host packs weights accordingly).
            # m=0 (k) and m=1 (v) run in one hid-arrival-paced kc loop so the
            # PE has 2x work per arriving hid tile while the 16MB hid load
            # streams in; m=2..5 then run at full speed on resident tiles.
            ps_k = [pr_ps.tile([P, 512], F32, tag=f"pp0_{n}", name="pp")
                    for n in range(NCH)]
            ps_v = [pr_ps.tile([P, 512], F32, tag=f"pp1_{n}", name="pp")
                    for n in range(NCH)]
            for kc in range(KCH):
                for n in range(NCH):
                    nc.tensor.matmul(
                        ps_k[n][:], wms[0][:, kc * P:(kc + 1) * P],
                        hts[kc][:, n * 512:(n + 1) * 512],
                        start=(kc == 0), stop=(kc == KCH - 1))
                for n in range(NCH):
                    nc.tensor.matmul(
                        ps_v[n][:], wms[1][:, kc * P:(kc + 1) * P],
                        hts[kc][:, n * 512:(n + 1) * 512],
                        start=(kc == 0), stop=(kc == KCH - 1))
            nc.sync.dma_start(wms[2][:], w_qkvT[2 * P:3 * P, :])
            for n in range(NCH):
                rope_chunk(k16, n, ps_k[n])
            for n in range(NCH):
                vtmp = rp_pool.tile([P, 512], BF, tag="vtmp",
                                    name="vtmp", bufs=2)
                nc.scalar.copy(vtmp[:], ps_v[n][:])
                for jj in range(4):
                    j = n * 4 + jj
                    nc.sync.dma_start_transpose(
                        vtok[j][:], vtmp[:, jj * P:(jj + 1) * P])
            for m in range(2, SLOTS):
                wm = wms[m]
                if m + 1 < SLOTS:
                    nc.sync.dma_start(wms[m + 1][:],
                                      w_qkvT[(m + 1) * P:(m + 2) * P, :])
                ps = [pr_ps.tile([P, 512], F32, tag=f"pp{m % 2}_{n}",
                                 name="pp") for n in range(NCH)]
                for kc in range(KCH):
                    for n in range(NCH):
                        nc.tensor.matmul(
                            ps[n][:], wm[:, kc * P:(kc + 1) * P],
                            hts[kc][:, n * 512:(n + 1) * 512],
                            start=(kc == 0), stop=(kc == KCH - 1))
                for n in range(NCH):
                    rope_chunk(q16[m - 2], n, ps[n])

            ph1.close()   # free hid/wq/cos/rope SBUF + qkv PSUM banks

            # ============ phase 2: attention + AG; o_proj after ====
            with tc.tile_pool(name="att", bufs=1) as att_pool, \
                 tc.tile_pool(name="ps2", bufs=1, space="PSUM") as ps2, \
                 tc.tile_pool(name="sm", bufs=2) as sm_pool, \
                 tc.tile_pool(name="ssum", bufs=1) as s_pool, \
                  tc.tile_pool(name="wo", bufs=1) as wo_pool, \
                 tc.tile_pool(name="oo", bufs=1) as oo_pool:

                wo_sb = wo_pool.tile([P, QH, KCH * P], BF, tag="wo")
                nc.sync.dma_start(
                    wo_sb[:], w_oT[:, :].rearrange("(m p) f -> p m f", p=P))

                # ag tiles share the att pool: the rotation WAR dependency
                # stops the scheduler from hoisting these sync-engine DMAs
                # (which block on AG completion) ahead of latency-critical
                # norm DMAs — head-of-line blocking on the in-order sync
                # engine cost 40us otherwise.
                def load_ag(c):
                    tiles = []
                    for kc in range(KCH):
                        at = att_pool.tile([P, 512], BF, tag="att", name="ag",
                                           bufs=88)
                        nc.sync.dma_start(
                            at[:], ag_out[c, kc * P:(kc + 1) * P, :])
                        tiles.append(at)
                    return tiles

                def load_ag_half(i):
                    tiles = []
                    for kc in range(KCH):
                        at = att_pool.tile([P, 512], BF, tag="att", name="agh",
                                           bufs=88)
                        nc.sync.dma_start(
                            at[:, :256], agh_out[i][kc * P:(kc + 1) * P, :])
                        tiles.append(at)
                    return tiles

                def mm_oproj(c, tiles, ms=range(QH), w=512, coff=0):
                    for m in ms:
                        pso = ps2.tile([P, 512], F32, tag="op", name="op",
                                       bufs=2)
                        for kc in range(KCH):
                            nc.tensor.matmul(
                                pso[:, :w], wo_sb[:, m, kc * P:(kc + 1) * P],
                                tiles[kc][:, :w],
                                start=(kc == 0), stop=(kc == KCH - 1))
                        osb = oo_pool.tile([P, 512], F32, tag="osb",
                                           name="osb", bufs=3)
                        nc.scalar.copy(osb[:, :w], pso[:, :w])
                        nc.sync.dma_start(
                            outT[m * P:(m + 1) * P,
                                 c * 512 + coff:c * 512 + coff + w],
                            osb[:, :w])

                # chunk order 1,2,3,0: the tail chain (last norm -> last AG
                # -> last o_proj) hangs off the TINY chunk 0 instead of the
                # big scalar-exp-paced chunk 3, and the big chunks' AGs and
                # o_proj overlap mid-run attention
                norm_pending = None
                ag_tiles = {}
                for idx, c in enumerate([2, 3, 1, 0]):
                    t0 = c * 512
                    jmax = 4 * c + 3
                    avp = [ps2.tile([P, 512], F32, tag=f"av{h}",
                                    name=f"av{h}") for h in range(QH)]
                    # softmax denominator accumulators (DVE, fp32)
                    S = [s_pool.tile([P, 512], F32, tag=f"s{h}",
                                     name=f"s{h}", bufs=2)
                         for h in range(QH)]
                    atts = {}

                    def scores(j, c=c, t0=t0, atts=atts):
                        toff = max(t0, j * P)
                        w = t0 + 512 - toff
                        for h in range(QH):
                            scp = ps2.tile([P, 512], F32, tag="sc",
                                           name="scp", bufs=2)
                            nc.tensor.matmul(
                                scp[:, :w], k16[:, j * P:(j + 1) * P],
                                q16[h][:, toff:toff + w],
                                start=True, stop=True)
                            att = att_pool.tile([P, 512], BF, tag="att",
                                                name="att", bufs=88)
                            nc.scalar.activation(att[:, :w], scp[:, :w], EXP,
                                                 scale=SCALE)
                            if j >= 4 * c:  # diagonal block: causal mask
                                nc.vector.tensor_mul(att[:, :P], att[:, :P],
                                                     tri_sb[:])
                            atts[(j, h)] = (att, toff, w)

                    def avdn(j, c=c, t0=t0, jmax=jmax, atts=atts, avp=avp,
                             S=S):
                        for h in range(QH):
                            att, toff, w = atts[(j, h)]
                            o = toff - t0
                            nc.tensor.matmul(
                                avp[h][:, o:o + w], vtok[j][:], att[:, :w],
                                start=(j == 0), stop=(j == jmax),
                                skip_group_check=True)
                        for h in range(QH):
                            att, toff, w = atts[(j, h)]
                            o = toff - t0
                            if j == 0:
                                nc.vector.tensor_copy(S[h][:], att[:])
                            else:
                                nc.vector.tensor_add(
                                    S[h][:, o:o + w], S[h][:, o:o + w],
                                    att[:, :w])

                    def make_norm(c=c, avp=avp, S=S):
                        dnrs = []
                        # issued at chunk end: s16 casts (DVE) feed the dn
                        # matmul without waiting; avp psum->SBUF copies
                        # (scalar) free the av banks so the next chunk's
                        # first AV matmul never waits on this chunk's norm
                        s16h = []
                        for h in range(QH):
                            s16 = sm_pool.tile([P, 512], BF, tag="s16",
                                               name="s16", bufs=4)
                            nc.vector.tensor_copy(s16[:], S[h][:])
                            s16h.append(s16)
                        avcp = []
                        for h in range(QH):
                            av_sb = sm_pool.tile([P, 512], F32, tag="avcp",
                                                 name="avcp", bufs=4)
                            nc.scalar.copy(av_sb[:], avp[h][:])
                            avcp.append(av_sb)

                        def norm_a():
                            # reciprocal-of-denominator pipeline head: ends
                            # in the dnr row DMAs so the PE-side bcp (in
                            # norm_b, two j-iterations later) never waits on
                            # the DMA roundtrip
                            dnp = ps2.tile([QH, 512], F32, tag="sc",
                                           name="dn", bufs=2)
                            for h in range(QH):
                                nc.tensor.matmul(
                                    dnp[:], onc4_sb[h][:], s16h[h][:],
                                    start=(h == 0), stop=(h == QH - 1),
                                    skip_group_check=True)
                            dn_sb = sm_pool.tile([QH, 512], F32, tag="dn_sb")
                            nc.vector.tensor_copy(dn_sb[:], dnp[:])
                            rc4 = sm_pool.tile([QH, 512], F32, tag="rc4")
                            scr = sm_pool.tile([QH, 512], F32, tag="scr")
                            nc.vector.reciprocal_approx_fast(rc4[:],
                                                             dn_sb[:])
                            rc16 = sm_pool.tile([QH, 512], BF, tag="rc16")
                            nc.vector.tensor_copy(rc16[:], rc4[:])
                            for h in range(QH):
                                dnr = sm_pool.tile([1, 512], BF, tag="dnr",
                                                   bufs=4)
                                nc.sync.dma_start(dnr[:], rc16[h:h + 1, :])
                                dnrs.append(dnr)

                        def norm_b():
                            for h in range(QH):
                                bcp = ps2.tile([P, 512], F32, tag="sc",
                                               name="bcp", bufs=2)
                                nc.tensor.matmul(bcp[:], onr_sb[:],
                                                 dnrs[h][:],
                                                 start=True, stop=True)
                                bc_sb = sm_pool.tile([P, 512], F32,
                                                     tag="bc_sb", bufs=3)
                                nc.vector.tensor_copy(bc_sb[:], bcp[:])
                                avn = sm_pool.tile([P, 512], BF, tag="avn",
                                                   bufs=4)
                                nc.vector.tensor_mul(avn[:], avcp[h][:],
                                                     bc_sb[:])
                                nc.sync.dma_start(
                                    ag_in[c, h * P:(h + 1) * P, :], avn[:])
                            nc.gpsimd.collective_compute(
                                "AllGather",
                                mybir.AluOpType.bypass,
                                replica_groups=[list(range(CORES))],
                                ins=[ag_in[c]],
                                outs=[ag_out[c]],
                            )
                        return norm_a, norm_b

                    # software pipeline: scores one j ahead; previous chunk's
                    # normalization + AllGather fire early; o_proj operand
                    # prefetch slots into the big attention chunk
                    scores(0)
                    for j in range(jmax + 1):
                        if j < jmax:
                            scores(j + 1)
                        if j == 0 and norm_pending is not None:
                            norm_pending[0]()
                        if j == 2 and norm_pending is not None:
                            norm_pending[1]()
                        if c == 3 and j == 10:
                            ag_tiles[2] = load_ag(2)
                        if c == 0 and j == 1:
                            ag_tiles[3] = load_ag(3)
                        avdn(j)
                    norm_pending = make_norm()
                norm_pending[0]()
                mm_oproj(2, ag_tiles[2], ms=[0])   # dnr DMAs land meanwhile
                norm_pending[1]()          # triggers AG(0)
                mm_oproj(2, ag_tiles[2], ms=[1, 2, 3])
                mm_oproj(3, ag_tiles[3])
                ag_tiles[1] = load_ag(1)
                mm_oproj(1, ag_tiles[1])
                ag_tiles[0] = load_ag(0)
                mm_oproj(0, ag_tiles[0])

            pstack.close()

    nc.finalize()
    return nc


_NC_CACHE = None


def _host_prep(positions, hidden_states, w_qkv, w_o):
    pos = np.asarray(positions).astype(np.float64)
    half = D // 2
    inv_freq = 1.0 / (THETA ** (np.arange(half, dtype=np.float64) * 2.0 / D))
    freqs = pos[:, None] * inv_freq[None, :]          # [T, 64]
    cos = np.cos(freqs).astype(np.float32).T          # [64, T]
    sin = np.sin(freqs).astype(np.float32).T
    cosT = np.concatenate([cos, cos], axis=0).astype(BF_NP)    # [128, T]
    sinT = np.concatenate([-sin, sin], axis=0).astype(BF_NP)   # sign fold
    hidT = np.ascontiguousarray(
        np.asarray(hidden_states, dtype=np.float32).T).astype(BF_NP)
    w_qkv = np.asarray(w_qkv, dtype=np.float32)
    w_o = np.asarray(w_o, dtype=np.float32)

    in_maps = []
    for c in range(CORES):
        rows = [
            w_qkv[H * D + c * D: H * D + (c + 1) * D],                  # k
            w_qkv[(H + KV) * D + c * D: (H + KV) * D + (c + 1) * D],    # v
            w_qkv[c * QH * D:(c + 1) * QH * D],                         # q0..3
        ]
        wcat = np.concatenate(rows, axis=0)             # [768, 4096] (out,in)
        # stationary-major: [m*128+p, kc*128+cc] = wcat[m*128+cc, kc*128+p]
        w_qkvT_c = np.ascontiguousarray(
            wcat.reshape(SLOTS, P, KCH, P).transpose(0, 3, 2, 1)
            .reshape(SLOTS * P, KCH * P)).astype(BF_NP)
        wo_slice = w_o[c * LOCAL:(c + 1) * LOCAL, :]    # [512 out, 4096 in]
        w_oT_c = np.ascontiguousarray(
            wo_slice.reshape(QH, P, KCH, P).transpose(0, 3, 2, 1)
            .reshape(QH * P, KCH * P)).astype(BF_NP)
        in_maps.append({
            "hidT": hidT,
            "w_qkvT": w_qkvT_c,
            "w_oT": w_oT_c,
            "cosT": cosT,
            "sinT": sinT,
        })
    return in_maps


def kernel(positions, hidden_states, w_qkv, w_o):
    global _NC_CACHE
    in_maps = _host_prep(positions, hidden_states, w_qkv, w_o)
    if _NC_CACHE is None:
        _NC_CACHE = build_nc()
    res = None
    for attempt in range(3):
        try:
            res = run_bass_kernel_spmd(_NC_CACHE, in_maps,
                                       core_ids=list(range(CORES)))
            break
        except Exception:
            if attempt == 2:
                raise
    outs = [res.results[c]["outT"].T for c in range(CORES)]   # [2048, 512]
    return np.ascontiguousarray(np.concatenate(outs, axis=1))
